# revision 20
# baseline (speedup 1.0000x reference)
"""GCN message passing (SpMM) on 8 Trainium2 NeuronCores.

out[r, :] = sum_{e: rows[e]==r} vals[e] * x[cols[e], :]

Sharding: 1D row partitioning. adj_rows is sorted, so core k owns output rows
[k*12500, (k+1)*12500) and the contiguous edge range hitting those rows.
No collectives; each core writes its own output slab.

Per-core algorithm (v5 = v4 windowed 4-bucket gather + slot-space output):
  - x is padded to [100000, 64] f32 (256B rows) and split into 4 node-range
    buckets of 25000 rows so dma_gather's int16 indices can address each.
  - Host greedily groups consecutive output rows into "windows" (<=32 rows,
    <=128 edges per bucket per window). Each (window, bucket) is one
    128-edge gather tile (tail-padded with zero-val edges).
  - All per-edge metadata (gather indices, vals, slot ids) is preloaded into
    SBUF once at kernel start, so the steady-state loop issues only:
    4 dma_gathers + 8 DVE ops + 120 matmuls + 1 ACT copy + 3 HWDGE output
    DMAs per 30-window chunk.
  - PE accumulates the 4 buckets' S^T @ G into one PSUM [32,48] slot per
    window => full segment sums.
  - v5 change vs v4: no dma_scatter_add.  PSUM chunk layout is chosen affine
    (window w_local = 10a + j -> psum partitions [32a,32a+32), free block j),
    so the chunk's 960 slots write to a slot-space DRAM tensor with 3 plain
    HWDGE dma_starts (zero Q7 descriptor-generation cost).  The host gathers
    row r from slot position 960*chunk + 320a + 32j + slot at unshard time
    (pure indexing).  This removes the scatter's Q7 work (~25% of runtime),
    the sidx metadata, the zero-slab preloads, and the 4-slab host sum.
  - Gathers run on SWDGE queues 0-3 (bucket b -> queue b) so descriptor
    generation uses all 4 queue contexts (8 Q7 cores); this is the kernel's
    bottleneck (~2.3ns per gather descriptor, 4 queues).
"""

import numpy as np

import concourse.bass as bass
import concourse.bacc as bacc
import concourse.mybir as mybir
import concourse.tile as tile
from concourse.bass_utils import run_bass_kernel_spmd

# ---------------- problem constants (hardcoded per the task contract) -------
N_NODES = 100000
D = 48
N_CORES = 8
R_PER_CORE = N_NODES // N_CORES  # 12500

# ---------------- kernel hyperparameters -----------------------------------
NB = 4               # node-range buckets (int16 gather indices: 25000 < 32768)
B_NODES = N_NODES // NB
EDGE_CAP = 128       # edges per (window, bucket) tile = PE contraction dim
SEG_CAP = 32         # max rows per window (= matmul M, psum partition group)
GP = 3               # usable 32-partition psum groups (offset 96 unusable)
CW = 30              # windows per chunk (= one PSUM bank: 3 groups x 10)
SC_H = CW // GP      # free blocks per psum bank (10)
EL = 64              # padded x row, f32 elements (256B)
SLOTS_PER_CHUNK = CW * SEG_CAP  # 960

_F32 = mybir.dt.float32
_I16 = mybir.dt.int16

_NIG = CW * EDGE_CAP          # gather indices per (chunk, bucket) = 3840
_GI_W = _NIG // 16            # 240 int16 per partition per chunk


def _wrap16(flat, reps=8):
    """[(n)] int16 -> [16*reps, n/16] in the 16-partition wrap, replicated."""
    n = flat.shape[0]
    w = flat.reshape(n // 16, 16).T  # [16, n/16]
    return np.tile(w, (reps, 1))


# ===========================================================================
# Host-side prep: pure index/layout transformation (no float math on data).
# ===========================================================================
def _bfd_pack(deg, n_win):
    """Worst-fit-decreasing (LPT balancing): assign rows to n_win windows
    (<=SEG_CAP rows, per-bucket degree sum <=EDGE_CAP).  Returns
    (win_of_row, slot_of_row) or None if infeasible at this n_win."""
    r_per_core = deg.shape[0]
    cap = np.full((n_win, NB), EDGE_CAP, np.int64)
    cnt = np.zeros(n_win, np.int64)
    win_of = np.empty(r_per_core, np.int64)
    slot_of = np.empty(r_per_core, np.int64)
    order = np.argsort(-deg.sum(1), kind="stable")
    big = 1 << 40
    for r in order:
        feas = (cnt < SEG_CAP) & (cap[:, 0] >= deg[r, 0]) \
            & (cap[:, 1] >= deg[r, 1]) & (cap[:, 2] >= deg[r, 2]) \
            & (cap[:, 3] >= deg[r, 3])
        slack = cap.sum(1) - np.where(feas, 0, big)
        w = int(np.argmax(slack))
        if not feas[w]:
            return None
        win_of[r] = w
        slot_of[r] = cnt[w]
        cap[w] -= deg[r]
        cnt[w] += 1
    return win_of, slot_of


def _pack_core(rows_l, cols, vals, r_per_core, n_win_target):
    n_e = rows_l.shape[0]
    bucket = (cols // B_NODES).astype(np.int64)
    col_loc = (cols - bucket * B_NODES).astype(np.int16)

    deg = np.zeros((r_per_core, NB), np.int64)
    np.add.at(deg, (rows_l, bucket), 1)
    assert deg.max() <= EDGE_CAP, "row degree exceeds tile capacity"

    n_win = n_win_target
    while True:
        packed = _bfd_pack(deg, n_win)
        if packed is not None:
            break
        n_win += CW
    window_of_row, slot_of_row = packed

    w_e = window_of_row[rows_l]
    slot_e = slot_of_row[rows_l].astype(np.float32)

    per_bucket = []
    for b in range(NB):
        sel = np.flatnonzero(bucket == b)
        o = np.argsort(w_e[sel], kind="stable")
        sel = sel[o]
        wb = w_e[sel]                       # non-decreasing after sort
        first = np.searchsorted(wb, np.arange(n_win))
        pos = np.arange(sel.shape[0]) - first[wb]
        assert pos.max(initial=0) < EDGE_CAP
        colb = np.zeros((n_win, EDGE_CAP), np.int16)
        valb = np.zeros((n_win, EDGE_CAP), np.float32)
        slotb = np.zeros((n_win, EDGE_CAP), np.float32)
        colb[wb, pos] = col_loc[sel]
        valb[wb, pos] = vals[sel]
        slotb[wb, pos] = slot_e[sel]
        per_bucket.append((colb, valb, slotb))

    # slot-space position of each local row: window w -> chunk c=w//30,
    # w_local = w%30 = 10a + j -> pos = 960c + 320a + 32j + slot
    wl = window_of_row % CW
    pos_of_row = (SLOTS_PER_CHUNK * (window_of_row // CW)
                  + 320 * (wl // SC_H) + SEG_CAP * (wl % SC_H) + slot_of_row)
    return per_bucket, pos_of_row, n_win


def prep_inputs(adj_rows, adj_cols, adj_vals):
    """Shard + pack. Returns (per-core in_map list, n_chunks, pos list)."""
    adj_rows = np.asarray(adj_rows).astype(np.int64)
    adj_cols = np.asarray(adj_cols).astype(np.int64)
    adj_vals = np.asarray(adj_vals).astype(np.float32)

    bounds = np.searchsorted(adj_rows, np.arange(N_CORES + 1) * R_PER_CORE)
    packed = []
    for k in range(N_CORES):
        e0, e1 = bounds[k], bounds[k + 1]
        rows_l = adj_rows[e0:e1] - k * R_PER_CORE
        # minimal chunk count that can hold this core's edges and rows
        n_win_target = CW * max(-(-int(e1 - e0) // (NB * _NIG)),
                                -(-R_PER_CORE // (SEG_CAP * CW)))
        packed.append(_pack_core(rows_l, adj_cols[e0:e1],
                                 adj_vals[e0:e1], R_PER_CORE, n_win_target))

    nw_max = max(p[2] for p in packed)
    nw_pad = -(-nw_max // CW) * CW
    n_chunks = nw_pad // CW

    iota = np.broadcast_to(np.arange(SEG_CAP, dtype=np.float32),
                           (128, SEG_CAP)).copy()
    in_maps = []
    pos_list = []
    for k in range(N_CORES):
        per_bucket, pos_of_row, n_win = packed[k]
        pos_list.append(pos_of_row)
        m = {"iota": iota}
        for b in range(NB):
            colb, valb, slotb = per_bucket[b]
            cb = np.zeros((nw_pad, EDGE_CAP), np.int16)
            vb = np.zeros((nw_pad, EDGE_CAP), np.float32)
            sb = np.zeros((nw_pad, EDGE_CAP), np.float32)
            cb[:n_win] = colb
            vb[:n_win] = valb
            sb[:n_win] = slotb
            # SBUF-resident layouts (one DMA each):
            # gidx: [128, n_chunks*_GI_W] int16 (16-wrap per chunk, x8)
            m[f"gidx{b}"] = np.concatenate([
                _wrap16(cb[c * CW:(c + 1) * CW].reshape(-1))
                for c in range(n_chunks)], axis=1)
            # vals/slot: [128, n_chunks*CW]; [p, c*CW+t] = edge t*128+p
            m[f"gval{b}"] = np.ascontiguousarray(
                vb.reshape(n_chunks, CW, EDGE_CAP).transpose(2, 0, 1)
                .reshape(128, n_chunks * CW))
            m[f"gslot{b}"] = np.ascontiguousarray(
                sb.reshape(n_chunks, CW, EDGE_CAP).transpose(2, 0, 1)
                .reshape(128, n_chunks * CW))
        in_maps.append(m)
    return in_maps, n_chunks, pos_list


def pad_x(x):
    x64 = np.zeros((N_NODES, EL), np.float32)
    x64[:, :D] = x
    return x64


# ===========================================================================
# Device program (shared across all 8 cores)
# ===========================================================================
def build_program(n_chunks, repeat=1, opts=None):
    opts = opts or {}
    nc = bacc.Bacc("TRN2", target_bir_lowering=False, debug=False,
                   num_devices=N_CORES, num_swdge_queues=4)
    x_d = nc.dram_tensor("x64", [N_NODES, EL], _F32, kind="ExternalInput")
    gidx_d = [nc.dram_tensor(f"gidx{b}", [128, n_chunks * _GI_W], _I16,
                             kind="ExternalInput") for b in range(NB)]
    gval_d = [nc.dram_tensor(f"gval{b}", [128, n_chunks * CW], _F32,
                             kind="ExternalInput") for b in range(NB)]
    gslot_d = [nc.dram_tensor(f"gslot{b}", [128, n_chunks * CW], _F32,
                              kind="ExternalInput") for b in range(NB)]
    iota_d = nc.dram_tensor("iota", [128, SEG_CAP], _F32,
                            kind="ExternalInput")
    out_d = nc.dram_tensor("out", [n_chunks * SLOTS_PER_CHUNK, D], _F32,
                           kind="ExternalOutput")

    with tile.TileContext(nc) as tc:
        with (
            tc.tile_pool(name="meta", bufs=1) as meta,
            tc.tile_pool(name="gbuf",
                         bufs=2 if opts.get("shallow") else 3) as gbuf,
            tc.tile_pool(name="sbuf_s", bufs=2) as sbuf_s,
            tc.tile_pool(name="sout", bufs=3) as sout,
            tc.tile_pool(name="psum", bufs=3 if opts.get("psum3") else 6,
                         space="PSUM") as psum,
        ):
            iota_t = meta.tile([128, SEG_CAP], _F32)
            # chunk-0 metadata in separate small tiles so the first gathers
            # don't wait for the full metadata load
            gi0, gv0, gs0 = [], [], []
            gi_all, gv_all, gs_all = [], [], []
            for b in range(NB):
                gi0_b = meta.tile([128, _GI_W], _I16, tag=f"gi0{b}")
                gv0_b = meta.tile([128, CW], _F32, tag=f"gv0{b}")
                gs0_b = meta.tile([128, CW], _F32, tag=f"gs0{b}")
                gi0.append(gi0_b)
                gv0.append(gv0_b)
                gs0.append(gs0_b)
                gi = meta.tile([128, (n_chunks - 1) * _GI_W], _I16,
                               tag=f"giA{b}")
                gv = meta.tile([128, (n_chunks - 1) * CW], _F32,
                               tag=f"gvA{b}")
                gs = meta.tile([128, (n_chunks - 1) * CW], _F32,
                               tag=f"gsA{b}")
                gi_all.append(gi)
                gv_all.append(gv)
                gs_all.append(gs)

            for _rep in range(repeat):
                nc.sync.dma_start(out=iota_t[:], in_=iota_d[:])
                for b in range(NB):
                    nc.sync.dma_start(out=gi0[b][:],
                                      in_=gidx_d[b][:, :_GI_W])
                    nc.sync.dma_start(out=gv0[b][:], in_=gval_d[b][:, :CW])
                    nc.sync.dma_start(out=gs0[b][:], in_=gslot_d[b][:, :CW])
                for b in range(NB):
                    nc.sync.dma_start(out=gi_all[b][:],
                                      in_=gidx_d[b][:, _GI_W:])
                    nc.sync.dma_start(out=gv_all[b][:], in_=gval_d[b][:, CW:])
                    nc.sync.dma_start(out=gs_all[b][:],
                                      in_=gslot_d[b][:, CW:])
                _chunk_loop(nc, n_chunks, x_d, out_d, iota_t,
                            (gi0, gv0, gs0), gi_all, gv_all,
                            gs_all, gbuf, sbuf_s, sout, psum, opts)
    nc.compile()
    return nc


def _chunk_loop(nc, n_chunks, x_d, out_d, iota_t, meta0, gi_all, gv_all,
                gs_all, gbuf, sbuf_s, sout, psum, opts):
    gi0, gv0, gs0 = meta0
    for c in range(n_chunks):
        g_ts, s_ts = [], []
        for b in range(NB):
            if c == 0:
                gi_c = gi0[b][:]
                gv_c = gv0[b][:]
                gs_c = gs0[b][:]
            else:
                gi_c = gi_all[b][:, (c - 1) * _GI_W:c * _GI_W]
                gv_c = gv_all[b][:, (c - 1) * CW:c * CW]
                gs_c = gs_all[b][:, (c - 1) * CW:c * CW]
            g_t = gbuf.tile([128, CW * EL], _F32, tag=f"g{b}")
            if not opts.get("no_gather"):
                nc.gpsimd.dma_gather(
                    out_ap=g_t[:].rearrange("p (t f) -> p t f", f=EL),
                    in_ap=x_d[B_NODES * b:B_NODES * (b + 1)],
                    idxs_ap=gi_c,
                    num_idxs=_NIG, num_idxs_reg=_NIG, elem_size=EL,
                    single_packet=False,
                    queue_num=0 if opts.get("one_q") else b,
                )
            g_ts.append(g_t)

            s_t = sbuf_s.tile([128, CW * SEG_CAP], _F32, tag=f"s{b}")
            if not opts.get("no_dve"):
                s3 = s_t[:].rearrange("p (t s) -> p t s", s=SEG_CAP)
                gs_b = gs_c.unsqueeze(2).to_broadcast([128, CW, SEG_CAP])
                io_b = iota_t[:].unsqueeze(1).to_broadcast(
                    [128, CW, SEG_CAP])
                gv_b = gv_c.unsqueeze(2).to_broadcast([128, CW, SEG_CAP])
                nc.vector.tensor_tensor(out=s3, in0=gs_b, in1=io_b,
                                        op=mybir.AluOpType.is_equal)
                nc.vector.tensor_tensor(out=s3, in0=s3, in1=gv_b,
                                        op=mybir.AluOpType.mult)
            s_ts.append(s_t)

        ps = psum.tile([128, SC_H * D], _F32, space="PSUM", tag="ps")
        if not opts.get("no_pe"):
            for wl in range(CW):
                a, j = wl // SC_H, wl % SC_H
                for b in range(NB):
                    nc.tensor.matmul(
                        out=ps[32 * a:32 * a + SEG_CAP,
                               D * j:D * j + D],
                        lhsT=s_ts[b][:, SEG_CAP * wl:SEG_CAP * (wl + 1)],
                        rhs=g_ts[b][:, EL * wl:EL * wl + D],
                        start=(b == 0), stop=(b == NB - 1),
                        skip_group_check=True,
                    )

        sc_t = sout.tile([128, SC_H * D], _F32, tag="sc")
        if not opts.get("no_pe"):
            nc.scalar.copy(out=sc_t[:96, :], in_=ps[:96, :])
        if not opts.get("no_out"):
            for a in range(GP):
                r0 = SLOTS_PER_CHUNK * c + 320 * a
                eng = nc.sync if (c * GP + a) % 2 == 0 else nc.scalar
                eng.dma_start(
                    out=out_d[r0:r0 + 320, :].rearrange(
                        "(j s) e -> s j e", s=SEG_CAP),
                    in_=sc_t[32 * a:32 * a + SEG_CAP, :].rearrange(
                        "p (j e) -> p j e", e=D),
                )


# ===========================================================================
# Entry point
# ===========================================================================
_CACHE = {}


def _get_program(n_chunks, repeat=1, opts=None):
    key = (n_chunks, repeat)
    if key not in _CACHE:
        _CACHE[key] = build_program(n_chunks, repeat, opts)
    return _CACHE[key]


def _run(adj_rows, adj_cols, adj_vals, x):
    x64 = pad_x(np.ascontiguousarray(np.asarray(x), dtype=np.float32))
    in_maps, n_chunks, pos_list = prep_inputs(adj_rows, adj_cols, adj_vals)
    for m in in_maps:
        m["x64"] = x64
    nc = _get_program(n_chunks)
    res = run_bass_kernel_spmd(nc, in_maps, core_ids=list(range(N_CORES)))
    out = np.empty((N_NODES, D), np.float32)
    for k in range(N_CORES):
        out[k * R_PER_CORE:(k + 1) * R_PER_CORE] = (
            res.results[k]["out"][pos_list[k], :])
    return out, res, (in_maps, n_chunks)


def kernel(adj_rows, adj_cols, adj_vals, x):
    out, _, _ = _run(adj_rows, adj_cols, adj_vals, x)
    return out


# revision 24
# speedup vs baseline: 1.0341x; 1.0341x over previous
"""GCN message passing (SpMM) on 8 Trainium2 NeuronCores.

out[r, :] = sum_{e: rows[e]==r} vals[e] * x[cols[e], :]

Sharding: 1D row partitioning. adj_rows is sorted, so core k owns output rows
[k*12500, (k+1)*12500) and the contiguous edge range hitting those rows.
No collectives; each core writes its own output slab.

Per-core algorithm (v5 = v4 windowed 4-bucket gather + slot-space output):
  - x is padded to [100000, 64] f32 (256B rows) and split into 4 node-range
    buckets of 25000 rows so dma_gather's int16 indices can address each.
  - Host greedily groups consecutive output rows into "windows" (<=32 rows,
    <=128 edges per bucket per window). Each (window, bucket) is one
    128-edge gather tile (tail-padded with zero-val edges).
  - All per-edge metadata (gather indices, vals, slot ids) is preloaded into
    SBUF once at kernel start, so the steady-state loop issues only:
    4 dma_gathers + 8 DVE ops + 120 matmuls + 1 ACT copy + 3 HWDGE output
    DMAs per 30-window chunk.
  - PE accumulates the 4 buckets' S^T @ G into one PSUM [32,48] slot per
    window => full segment sums.
  - v5 change vs v4: no dma_scatter_add.  PSUM chunk layout is chosen affine
    (window w_local = 10a + j -> psum partitions [32a,32a+32), free block j),
    so the chunk's 960 slots write to a slot-space DRAM tensor with 3 plain
    HWDGE dma_starts (zero Q7 descriptor-generation cost).  The host gathers
    row r from slot position 960*chunk + 320a + 32j + slot at unshard time
    (pure indexing).  This removes the scatter's Q7 work (~25% of runtime),
    the sidx metadata, the zero-slab preloads, and the 4-slab host sum.
  - Gathers run on SWDGE queues 0-3 (bucket b -> queue b) so descriptor
    generation uses all 4 queue contexts (8 Q7 cores); this is the kernel's
    bottleneck (~2.3ns per gather descriptor, 4 queues).
"""

import numpy as np

import concourse.bass as bass
import concourse.bacc as bacc
import concourse.mybir as mybir
import concourse.tile as tile
from concourse.bass_utils import run_bass_kernel_spmd

# ---------------- problem constants (hardcoded per the task contract) -------
N_NODES = 100000
D = 48
N_CORES = 8
R_PER_CORE = N_NODES // N_CORES  # 12500

# ---------------- kernel hyperparameters -----------------------------------
NB = 4               # node-range buckets (int16 gather indices: 25000 < 32768)
B_NODES = N_NODES // NB
EDGE_CAP = 128       # edges per (window, bucket) tile = PE contraction dim
SEG_CAP = 32         # max rows per window (= matmul M, psum partition group)
GP = 3               # usable 32-partition psum groups (offset 96 unusable)
CW = 30              # windows per chunk (= one PSUM bank: 3 groups x 10)
SC_H = CW // GP      # free blocks per psum bank (10)
EL = 64              # padded x row, f32 elements (256B)
SLOTS_PER_CHUNK = CW * SEG_CAP  # 960

_F32 = mybir.dt.float32
_I16 = mybir.dt.int16

_NIG = CW * EDGE_CAP          # gather indices per (chunk, bucket) = 3840
_GI_W = _NIG // 16            # 240 int16 per partition per chunk


def _wrap16(flat, reps=8):
    """[(n)] int16 -> [16*reps, n/16] in the 16-partition wrap, replicated."""
    n = flat.shape[0]
    w = flat.reshape(n // 16, 16).T  # [16, n/16]
    return np.tile(w, (reps, 1))


# ===========================================================================
# Host-side prep: pure index/layout transformation (no float math on data).
# ===========================================================================
def _bfd_pack(deg, n_win, forbid=None):
    """Worst-fit-decreasing (LPT balancing): assign rows to n_win windows
    (<=SEG_CAP rows, per-bucket degree sum <=EDGE_CAP).  Returns
    (win_of_row, slot_of_row) or None if infeasible at this n_win.
    Windows marked in `forbid` receive no rows (kept empty so their gather
    tiles can be runtime-trimmed)."""
    r_per_core = deg.shape[0]
    cap = np.full((n_win, NB), EDGE_CAP, np.int64)
    cnt = np.zeros(n_win, np.int64)
    if forbid is not None:
        cnt[forbid] = SEG_CAP
    win_of = np.empty(r_per_core, np.int64)
    slot_of = np.empty(r_per_core, np.int64)
    order = np.argsort(-deg.sum(1), kind="stable")
    big = 1 << 40
    for r in order:
        feas = (cnt < SEG_CAP) & (cap[:, 0] >= deg[r, 0]) \
            & (cap[:, 1] >= deg[r, 1]) & (cap[:, 2] >= deg[r, 2]) \
            & (cap[:, 3] >= deg[r, 3])
        slack = cap.sum(1) - np.where(feas, 0, big)
        w = int(np.argmax(slack))
        if not feas[w]:
            return None
        win_of[r] = w
        slot_of[r] = cnt[w]
        cap[w] -= deg[r]
        cnt[w] += 1
    return win_of, slot_of


def _pack_core(rows_l, cols, vals, r_per_core, n_win_target):
    n_e = rows_l.shape[0]
    bucket = (cols // B_NODES).astype(np.int64)
    col_loc = (cols - bucket * B_NODES).astype(np.int16)

    deg = np.zeros((r_per_core, NB), np.int64)
    np.add.at(deg, (rows_l, bucket), 1)
    assert deg.max() <= EDGE_CAP, "row degree exceeds tile capacity"

    n_win = n_win_target
    spill = True
    while True:
        forbid = (np.arange(n_win) % CW == CW - 1) if spill else None
        packed = _bfd_pack(deg, n_win, forbid)
        if packed is not None:
            break
        if spill:
            spill = False       # retry same n_win without spill reservation
        else:
            n_win += CW
    window_of_row, slot_of_row = packed
    used = np.zeros(n_win, bool)
    used[window_of_row] = True

    w_e = window_of_row[rows_l]
    slot_e = slot_of_row[rows_l].astype(np.float32)

    per_bucket = []
    for b in range(NB):
        sel = np.flatnonzero(bucket == b)
        o = np.argsort(w_e[sel], kind="stable")
        sel = sel[o]
        wb = w_e[sel]                       # non-decreasing after sort
        first = np.searchsorted(wb, np.arange(n_win))
        pos = np.arange(sel.shape[0]) - first[wb]
        assert pos.max(initial=0) < EDGE_CAP
        colb = np.zeros((n_win, EDGE_CAP), np.int16)
        valb = np.zeros((n_win, EDGE_CAP), np.float32)
        slotb = np.zeros((n_win, EDGE_CAP), np.float32)
        colb[wb, pos] = col_loc[sel]
        valb[wb, pos] = vals[sel]
        slotb[wb, pos] = slot_e[sel]
        per_bucket.append((colb, valb, slotb))

    # slot-space position of each local row: window w -> chunk c=w//30,
    # w_local = w%30 = 10a + j -> pos = 960c + 320a + 32j + slot
    wl = window_of_row % CW
    pos_of_row = (SLOTS_PER_CHUNK * (window_of_row // CW)
                  + 320 * (wl // SC_H) + SEG_CAP * (wl % SC_H) + slot_of_row)
    return per_bucket, pos_of_row, n_win, used


def prep_inputs(adj_rows, adj_cols, adj_vals):
    """Shard + pack. Returns (per-core in_map list, n_chunks, pos list)."""
    adj_rows = np.asarray(adj_rows).astype(np.int64)
    adj_cols = np.asarray(adj_cols).astype(np.int64)
    adj_vals = np.asarray(adj_vals).astype(np.float32)

    bounds = np.searchsorted(adj_rows, np.arange(N_CORES + 1) * R_PER_CORE)
    packed = []
    for k in range(N_CORES):
        e0, e1 = bounds[k], bounds[k + 1]
        rows_l = adj_rows[e0:e1] - k * R_PER_CORE
        # minimal chunk count that can hold this core's edges and rows
        n_win_target = CW * max(-(-int(e1 - e0) // (NB * _NIG)),
                                -(-R_PER_CORE // (SEG_CAP * CW)))
        packed.append(_pack_core(rows_l, adj_cols[e0:e1],
                                 adj_vals[e0:e1], R_PER_CORE, n_win_target))

    nw_max = max(p[2] for p in packed)
    nw_pad = -(-nw_max // CW) * CW
    n_chunks = nw_pad // CW

    iota = np.broadcast_to(np.arange(SEG_CAP, dtype=np.float32),
                           (128, SEG_CAP)).copy()
    in_maps = []
    pos_list = []
    for k in range(N_CORES):
        per_bucket, pos_of_row, n_win, used = packed[k]
        pos_list.append(pos_of_row)
        # windows forming a trailing empty run within their chunk's gather
        # stream: mark their gather indices -1 so the ucode's trailing-
        # negative trim skips their descriptors entirely.
        used_pad = np.zeros(nw_pad, bool)
        used_pad[:n_win] = used
        trim = np.zeros(nw_pad, bool)
        for c in range(n_chunks):
            for w in range(CW * c + CW - 1, CW * c - 1, -1):
                if used_pad[w]:
                    break
                trim[w] = True
        m = {"iota": iota}
        for b in range(NB):
            colb, valb, slotb = per_bucket[b]
            cb = np.zeros((nw_pad, EDGE_CAP), np.int16)
            vb = np.zeros((nw_pad, EDGE_CAP), np.float32)
            sb = np.zeros((nw_pad, EDGE_CAP), np.float32)
            cb[:n_win] = colb
            vb[:n_win] = valb
            sb[:n_win] = slotb
            cb[trim] = -1
            # SBUF-resident layouts (one DMA each):
            # gidx: [128, n_chunks*_GI_W] int16 (16-wrap per chunk, x8)
            m[f"gidx{b}"] = np.concatenate([
                _wrap16(cb[c * CW:(c + 1) * CW].reshape(-1))
                for c in range(n_chunks)], axis=1)
            # vals/slot: [128, n_chunks*CW]; [p, c*CW+t] = edge t*128+p
            m[f"gval{b}"] = np.ascontiguousarray(
                vb.reshape(n_chunks, CW, EDGE_CAP).transpose(2, 0, 1)
                .reshape(128, n_chunks * CW))
            m[f"gslot{b}"] = np.ascontiguousarray(
                sb.reshape(n_chunks, CW, EDGE_CAP).transpose(2, 0, 1)
                .reshape(128, n_chunks * CW))
        in_maps.append(m)
    return in_maps, n_chunks, pos_list


def pad_x(x):
    x64 = np.zeros((N_NODES, EL), np.float32)
    x64[:, :D] = x
    return x64


# ===========================================================================
# Device program (shared across all 8 cores)
# ===========================================================================
def build_program(n_chunks, repeat=1, opts=None):
    opts = opts or {}
    nc = bacc.Bacc("TRN2", target_bir_lowering=False, debug=False,
                   num_devices=N_CORES, num_swdge_queues=4)
    x_d = nc.dram_tensor("x64", [N_NODES, EL], _F32, kind="ExternalInput")
    gidx_d = [nc.dram_tensor(f"gidx{b}", [128, n_chunks * _GI_W], _I16,
                             kind="ExternalInput") for b in range(NB)]
    gval_d = [nc.dram_tensor(f"gval{b}", [128, n_chunks * CW], _F32,
                             kind="ExternalInput") for b in range(NB)]
    gslot_d = [nc.dram_tensor(f"gslot{b}", [128, n_chunks * CW], _F32,
                              kind="ExternalInput") for b in range(NB)]
    iota_d = nc.dram_tensor("iota", [128, SEG_CAP], _F32,
                            kind="ExternalInput")
    out_d = nc.dram_tensor("out", [n_chunks * SLOTS_PER_CHUNK, D], _F32,
                           kind="ExternalOutput")

    with tile.TileContext(nc) as tc:
        with (
            tc.tile_pool(name="meta", bufs=1) as meta,
            tc.tile_pool(name="gbuf",
                         bufs=2 if opts.get("shallow") else 3) as gbuf,
            tc.tile_pool(name="sbuf_s", bufs=2) as sbuf_s,
            tc.tile_pool(name="sout", bufs=3) as sout,
            tc.tile_pool(name="psum", bufs=3 if opts.get("psum3") else 6,
                         space="PSUM") as psum,
        ):
            iota_t = meta.tile([128, SEG_CAP], _F32)
            # chunk-0 metadata in separate small tiles so the first gathers
            # don't wait for the full metadata load
            gi0, gv0, gs0 = [], [], []
            gi_all, gv_all, gs_all = [], [], []
            for b in range(NB):
                gi0_b = meta.tile([128, _GI_W], _I16, tag=f"gi0{b}")
                gv0_b = meta.tile([128, CW], _F32, tag=f"gv0{b}")
                gs0_b = meta.tile([128, CW], _F32, tag=f"gs0{b}")
                gi0.append(gi0_b)
                gv0.append(gv0_b)
                gs0.append(gs0_b)
                gi = meta.tile([128, (n_chunks - 1) * _GI_W], _I16,
                               tag=f"giA{b}")
                gv = meta.tile([128, (n_chunks - 1) * CW], _F32,
                               tag=f"gvA{b}")
                gs = meta.tile([128, (n_chunks - 1) * CW], _F32,
                               tag=f"gsA{b}")
                gi_all.append(gi)
                gv_all.append(gv)
                gs_all.append(gs)

            for _rep in range(repeat):
                nc.sync.dma_start(out=iota_t[:], in_=iota_d[:])
                for b in range(NB):
                    nc.sync.dma_start(out=gi0[b][:],
                                      in_=gidx_d[b][:, :_GI_W])
                    nc.sync.dma_start(out=gv0[b][:], in_=gval_d[b][:, :CW])
                    nc.sync.dma_start(out=gs0[b][:], in_=gslot_d[b][:, :CW])
                for b in range(NB):
                    nc.sync.dma_start(out=gi_all[b][:],
                                      in_=gidx_d[b][:, _GI_W:])
                    nc.sync.dma_start(out=gv_all[b][:], in_=gval_d[b][:, CW:])
                    nc.sync.dma_start(out=gs_all[b][:],
                                      in_=gslot_d[b][:, CW:])
                _chunk_loop(nc, n_chunks, x_d, out_d, iota_t,
                            (gi0, gv0, gs0), gi_all, gv_all,
                            gs_all, gbuf, sbuf_s, sout, psum, opts)
    nc.compile()
    return nc


def _chunk_loop(nc, n_chunks, x_d, out_d, iota_t, meta0, gi_all, gv_all,
                gs_all, gbuf, sbuf_s, sout, psum, opts):
    gi0, gv0, gs0 = meta0
    for c in range(n_chunks):
        g_ts, s_ts = [], []
        for b in range(NB):
            if c == 0:
                gi_c = gi0[b][:]
                gv_c = gv0[b][:]
                gs_c = gs0[b][:]
            else:
                gi_c = gi_all[b][:, (c - 1) * _GI_W:c * _GI_W]
                gv_c = gv_all[b][:, (c - 1) * CW:c * CW]
                gs_c = gs_all[b][:, (c - 1) * CW:c * CW]
            g_t = gbuf.tile([128, CW * EL], _F32, tag=f"g{b}")
            if not opts.get("no_gather"):
                nc.gpsimd.dma_gather(
                    out_ap=g_t[:].rearrange("p (t f) -> p t f", f=EL),
                    in_ap=x_d[B_NODES * b:B_NODES * (b + 1)],
                    idxs_ap=gi_c,
                    num_idxs=_NIG, num_idxs_reg=_NIG, elem_size=EL,
                    single_packet=False,
                    queue_num=0 if opts.get("one_q") else b,
                )
            g_ts.append(g_t)

            s_t = sbuf_s.tile([128, CW * SEG_CAP], _F32, tag=f"s{b}")
            if not opts.get("no_dve"):
                s3 = s_t[:].rearrange("p (t s) -> p t s", s=SEG_CAP)
                gs_b = gs_c.unsqueeze(2).to_broadcast([128, CW, SEG_CAP])
                io_b = iota_t[:].unsqueeze(1).to_broadcast(
                    [128, CW, SEG_CAP])
                gv_b = gv_c.unsqueeze(2).to_broadcast([128, CW, SEG_CAP])
                nc.vector.tensor_tensor(out=s3, in0=gs_b, in1=io_b,
                                        op=mybir.AluOpType.is_equal)
                nc.vector.tensor_tensor(out=s3, in0=s3, in1=gv_b,
                                        op=mybir.AluOpType.mult)
            s_ts.append(s_t)

        ps = psum.tile([128, SC_H * D], _F32, space="PSUM", tag="ps")
        if not opts.get("no_pe"):
            for wl in range(CW):
                a, j = wl // SC_H, wl % SC_H
                for b in range(NB):
                    nc.tensor.matmul(
                        out=ps[32 * a:32 * a + SEG_CAP,
                               D * j:D * j + D],
                        lhsT=s_ts[b][:, SEG_CAP * wl:SEG_CAP * (wl + 1)],
                        rhs=g_ts[b][:, EL * wl:EL * wl + D],
                        start=(b == 0), stop=(b == NB - 1),
                        skip_group_check=True,
                    )

        sc_t = sout.tile([128, SC_H * D], _F32, tag="sc")
        if not opts.get("no_pe"):
            nc.scalar.copy(out=sc_t[:96, :], in_=ps[:96, :])
        if not opts.get("no_out"):
            for a in range(GP):
                r0 = SLOTS_PER_CHUNK * c + 320 * a
                eng = nc.sync if (c * GP + a) % 2 == 0 else nc.scalar
                eng.dma_start(
                    out=out_d[r0:r0 + 320, :].rearrange(
                        "(j s) e -> s j e", s=SEG_CAP),
                    in_=sc_t[32 * a:32 * a + SEG_CAP, :].rearrange(
                        "p (j e) -> p j e", e=D),
                )


# ===========================================================================
# Entry point
# ===========================================================================
_CACHE = {}


def _get_program(n_chunks, repeat=1, opts=None):
    key = (n_chunks, repeat)
    if key not in _CACHE:
        _CACHE[key] = build_program(n_chunks, repeat, opts)
    return _CACHE[key]


def _run(adj_rows, adj_cols, adj_vals, x):
    x64 = pad_x(np.ascontiguousarray(np.asarray(x), dtype=np.float32))
    in_maps, n_chunks, pos_list = prep_inputs(adj_rows, adj_cols, adj_vals)
    for m in in_maps:
        m["x64"] = x64
    nc = _get_program(n_chunks)
    res = run_bass_kernel_spmd(nc, in_maps, core_ids=list(range(N_CORES)))
    out = np.empty((N_NODES, D), np.float32)
    for k in range(N_CORES):
        out[k * R_PER_CORE:(k + 1) * R_PER_CORE] = (
            res.results[k]["out"][pos_list[k], :])
    return out, res, (in_maps, n_chunks)


def kernel(adj_rows, adj_cols, adj_vals, x):
    out, _, _ = _run(adj_rows, adj_cols, adj_vals, x)
    return out


# revision 29
# speedup vs baseline: 1.0620x; 1.0270x over previous
"""GCN message passing (SpMM) on 8 Trainium2 NeuronCores.

out[r, :] = sum_{e: rows[e]==r} vals[e] * x[cols[e], :]

Sharding: 1D row partitioning. adj_rows is sorted, so core k owns output rows
[k*12500, (k+1)*12500) and the contiguous edge range hitting those rows.
No collectives; each core writes its own output slab.

Per-core algorithm (v5 = v4 windowed 4-bucket gather + slot-space output):
  - x is padded to [100000, 64] f32 (256B rows) and split into 4 node-range
    buckets of 25000 rows so dma_gather's int16 indices can address each.
  - Host packs output rows into "windows" (<=32 rows, <=128 edges per
    bucket per window) with a worst-fit-decreasing heuristic over the 4
    per-bucket degree dims; rows are permuted freely since the output is
    gathered from slot space host-side.  This reaches the minimal 14 chunks
    (420 windows) vs ~450 for consecutive-row grouping.  Each (window,
    bucket) is one 128-edge gather tile (tail-padded with zero-val edges).
    The last window of each chunk is kept empty when feasible; its all-pad
    tiles sit at the tail of each (bucket, chunk) index stream and are
    marked idx=-1, which dma_gather's trailing-negative trim skips at
    runtime (no descriptors generated).
  - All per-edge metadata (gather indices, vals, slot ids) is preloaded into
    SBUF once at kernel start, so the steady-state loop issues only:
    4 dma_gathers + 8 DVE ops + 120 matmuls + 1 ACT copy + 3 HWDGE output
    DMAs per 30-window chunk.
  - PE accumulates the 4 buckets' S^T @ G into one PSUM [32,48] slot per
    window => full segment sums.
  - v5 change vs v4: no dma_scatter_add.  PSUM chunk layout is chosen affine
    (window w_local = 10a + j -> psum partitions [32a,32a+32), free block j),
    so the chunk's 960 slots write to a slot-space DRAM tensor with 3 plain
    HWDGE dma_starts (zero Q7 descriptor-generation cost).  The host gathers
    row r from slot position 960*chunk + 320a + 32j + slot at unshard time
    (pure indexing).  This removes the scatter's Q7 work (~25% of runtime),
    the sidx metadata, the zero-slab preloads, and the 4-slab host sum.
  - Gathers run on SWDGE queues 0-3 (bucket b -> queue b) so descriptor
    generation uses all 4 queue contexts (8 Q7 cores); this is the kernel's
    bottleneck (~2.3ns per gather descriptor, 4 queues).
"""

import numpy as np

import concourse.bass as bass
import concourse.bacc as bacc
import concourse.mybir as mybir
import concourse.tile as tile
from concourse.bass_utils import run_bass_kernel_spmd

# ---------------- problem constants (hardcoded per the task contract) -------
N_NODES = 100000
D = 48
N_CORES = 8
R_PER_CORE = N_NODES // N_CORES  # 12500

# ---------------- kernel hyperparameters -----------------------------------
NB = 4               # node-range buckets (int16 gather indices: 25000 < 32768)
B_NODES = N_NODES // NB
EDGE_CAP = 128       # edges per (window, bucket) tile = PE contraction dim
SEG_CAP = 32         # max rows per window (= matmul M, psum partition group)
GP = 3               # usable 32-partition psum groups (offset 96 unusable)
CW = 30              # windows per chunk (= one PSUM bank: 3 groups x 10)
SC_H = CW // GP      # free blocks per psum bank (10)
EL = 64              # padded x row, f32 elements (256B)
SLOTS_PER_CHUNK = CW * SEG_CAP  # 960

_F32 = mybir.dt.float32
_I16 = mybir.dt.int16

_NIG = CW * EDGE_CAP          # gather indices per (chunk, bucket) = 3840
_GI_W = _NIG // 16            # 240 int16 per partition per chunk


def _wrap16(flat, reps=8):
    """[(n)] int16 -> [16*reps, n/16] in the 16-partition wrap, replicated."""
    n = flat.shape[0]
    w = flat.reshape(n // 16, 16).T  # [16, n/16]
    return np.tile(w, (reps, 1))


# ===========================================================================
# Host-side prep: pure index/layout transformation (no float math on data).
# ===========================================================================
def _bfd_pack(deg, n_win, forbid=None):
    """Worst-fit-decreasing (LPT balancing): assign rows to n_win windows
    (<=SEG_CAP rows, per-bucket degree sum <=EDGE_CAP).  Returns
    (win_of_row, slot_of_row) or None if infeasible at this n_win.
    Windows marked in `forbid` receive no rows (kept empty so their gather
    tiles can be runtime-trimmed)."""
    r_per_core = deg.shape[0]
    cap = np.full((n_win, NB), EDGE_CAP, np.int64)
    cnt = np.zeros(n_win, np.int64)
    if forbid is not None:
        cnt[forbid] = SEG_CAP
    win_of = np.empty(r_per_core, np.int64)
    slot_of = np.empty(r_per_core, np.int64)
    order = np.argsort(-deg.sum(1), kind="stable")
    big = 1 << 40
    for r in order:
        feas = (cnt < SEG_CAP) & (cap[:, 0] >= deg[r, 0]) \
            & (cap[:, 1] >= deg[r, 1]) & (cap[:, 2] >= deg[r, 2]) \
            & (cap[:, 3] >= deg[r, 3])
        slack = cap.sum(1) - np.where(feas, 0, big)
        w = int(np.argmax(slack))
        if not feas[w]:
            return None
        win_of[r] = w
        slot_of[r] = cnt[w]
        cap[w] -= deg[r]
        cnt[w] += 1
    return win_of, slot_of


def _pack_core(rows_l, cols, vals, r_per_core, n_win_target):
    n_e = rows_l.shape[0]
    bucket = (cols // B_NODES).astype(np.int64)
    col_loc = (cols - bucket * B_NODES).astype(np.int16)

    deg = np.zeros((r_per_core, NB), np.int64)
    np.add.at(deg, (rows_l, bucket), 1)
    assert deg.max() <= EDGE_CAP, "row degree exceeds tile capacity"

    n_win = n_win_target
    spill = True
    while True:
        forbid = (np.arange(n_win) % CW == CW - 1) if spill else None
        packed = _bfd_pack(deg, n_win, forbid)
        if packed is not None:
            break
        if spill:
            spill = False       # retry same n_win without spill reservation
        else:
            n_win += CW
    window_of_row, slot_of_row = packed
    used = np.zeros(n_win, bool)
    used[window_of_row] = True

    w_e = window_of_row[rows_l]
    slot_e = slot_of_row[rows_l].astype(np.float32)

    per_bucket = []
    for b in range(NB):
        sel = np.flatnonzero(bucket == b)
        o = np.argsort(w_e[sel], kind="stable")
        sel = sel[o]
        wb = w_e[sel]                       # non-decreasing after sort
        first = np.searchsorted(wb, np.arange(n_win))
        pos = np.arange(sel.shape[0]) - first[wb]
        assert pos.max(initial=0) < EDGE_CAP
        colb = np.zeros((n_win, EDGE_CAP), np.int16)
        valb = np.zeros((n_win, EDGE_CAP), np.float32)
        slotb = np.zeros((n_win, EDGE_CAP), np.float32)
        colb[wb, pos] = col_loc[sel]
        valb[wb, pos] = vals[sel]
        slotb[wb, pos] = slot_e[sel]
        per_bucket.append((colb, valb, slotb))

    # slot-space position of each local row: window w -> chunk c=w//30,
    # w_local = w%30 = 10a + j -> pos = 960c + 320a + 32j + slot
    wl = window_of_row % CW
    pos_of_row = (SLOTS_PER_CHUNK * (window_of_row // CW)
                  + 320 * (wl // SC_H) + SEG_CAP * (wl % SC_H) + slot_of_row)
    return per_bucket, pos_of_row, n_win, used


def prep_inputs(adj_rows, adj_cols, adj_vals):
    """Shard + pack. Returns (per-core in_map list, n_chunks, pos list)."""
    adj_rows = np.asarray(adj_rows).astype(np.int64)
    adj_cols = np.asarray(adj_cols).astype(np.int64)
    adj_vals = np.asarray(adj_vals).astype(np.float32)

    bounds = np.searchsorted(adj_rows, np.arange(N_CORES + 1) * R_PER_CORE)
    packed = []
    for k in range(N_CORES):
        e0, e1 = bounds[k], bounds[k + 1]
        rows_l = adj_rows[e0:e1] - k * R_PER_CORE
        # minimal chunk count that can hold this core's edges and rows
        n_win_target = CW * max(-(-int(e1 - e0) // (NB * _NIG)),
                                -(-R_PER_CORE // (SEG_CAP * CW)))
        packed.append(_pack_core(rows_l, adj_cols[e0:e1],
                                 adj_vals[e0:e1], R_PER_CORE, n_win_target))

    nw_max = max(p[2] for p in packed)
    nw_pad = -(-nw_max // CW) * CW
    n_chunks = nw_pad // CW

    iota = np.broadcast_to(np.arange(SEG_CAP, dtype=np.float32),
                           (128, SEG_CAP)).copy()
    in_maps = []
    pos_list = []
    for k in range(N_CORES):
        per_bucket, pos_of_row, n_win, used = packed[k]
        pos_list.append(pos_of_row)
        # windows forming a trailing empty run within their chunk's gather
        # stream: mark their gather indices -1 so the ucode's trailing-
        # negative trim skips their descriptors entirely.
        used_pad = np.zeros(nw_pad, bool)
        used_pad[:n_win] = used
        trim = np.zeros(nw_pad, bool)
        for c in range(n_chunks):
            for w in range(CW * c + CW - 1, CW * c - 1, -1):
                if used_pad[w]:
                    break
                trim[w] = True
        m = {"iota": iota}
        for b in range(NB):
            colb, valb, slotb = per_bucket[b]
            cb = np.zeros((nw_pad, EDGE_CAP), np.int16)
            vb = np.zeros((nw_pad, EDGE_CAP), np.float32)
            sb = np.zeros((nw_pad, EDGE_CAP), np.float32)
            cb[:n_win] = colb
            vb[:n_win] = valb
            sb[:n_win] = slotb
            cb[trim] = -1
            # SBUF-resident layouts (one DMA each):
            # gidx: [128, n_chunks*_GI_W] int16 (16-wrap per chunk, x8)
            m[f"gidx{b}"] = np.concatenate([
                _wrap16(cb[c * CW:(c + 1) * CW].reshape(-1))
                for c in range(n_chunks)], axis=1)
            # vals/slot: [128, n_chunks*CW]; [p, c*CW+t] = edge t*128+p
            m[f"gval{b}"] = np.ascontiguousarray(
                vb.reshape(n_chunks, CW, EDGE_CAP).transpose(2, 0, 1)
                .reshape(128, n_chunks * CW))
            m[f"gslot{b}"] = np.ascontiguousarray(
                sb.reshape(n_chunks, CW, EDGE_CAP).transpose(2, 0, 1)
                .reshape(128, n_chunks * CW))
        in_maps.append(m)
    return in_maps, n_chunks, pos_list


def pad_x(x):
    x64 = np.zeros((N_NODES, EL), np.float32)
    x64[:, :D] = x
    return x64


# ===========================================================================
# Device program (shared across all 8 cores)
# ===========================================================================
def build_program(n_chunks, repeat=1, opts=None):
    opts = opts or {}
    nc = bacc.Bacc("TRN2", target_bir_lowering=False, debug=False,
                   num_devices=N_CORES, num_swdge_queues=4)
    x_d = nc.dram_tensor("x64", [N_NODES, EL], _F32, kind="ExternalInput")
    gidx_d = [nc.dram_tensor(f"gidx{b}", [128, n_chunks * _GI_W], _I16,
                             kind="ExternalInput") for b in range(NB)]
    gval_d = [nc.dram_tensor(f"gval{b}", [128, n_chunks * CW], _F32,
                             kind="ExternalInput") for b in range(NB)]
    gslot_d = [nc.dram_tensor(f"gslot{b}", [128, n_chunks * CW], _F32,
                              kind="ExternalInput") for b in range(NB)]
    iota_d = nc.dram_tensor("iota", [128, SEG_CAP], _F32,
                            kind="ExternalInput")
    out_d = nc.dram_tensor("out", [n_chunks * SLOTS_PER_CHUNK, D], _F32,
                           kind="ExternalOutput")

    with tile.TileContext(nc) as tc:
        with (
            tc.tile_pool(name="meta", bufs=1) as meta,
            tc.tile_pool(name="gbuf",
                         bufs=2 if (opts.get("shallow") or opts.get("sc2"))
                         else 3) as gbuf,
            tc.tile_pool(name="sbuf_s", bufs=2) as sbuf_s,
            tc.tile_pool(name="sout", bufs=2) as sout,
            tc.tile_pool(name="psum", bufs=2, space="PSUM") as psum,
        ):
            iota_t = meta.tile([128, SEG_CAP], _F32)
            # chunk-0 metadata in separate small tiles so the first gathers
            # don't wait for the full metadata load
            gi0, gv0, gs0 = [], [], []
            gi_all, gv_all, gs_all = [], [], []
            for b in range(NB):
                gi0_b = meta.tile([128, _GI_W], _I16, tag=f"gi0{b}")
                gv0_b = meta.tile([128, CW], _F32, tag=f"gv0{b}")
                gs0_b = meta.tile([128, CW], _F32, tag=f"gs0{b}")
                gi0.append(gi0_b)
                gv0.append(gv0_b)
                gs0.append(gs0_b)
                gi = meta.tile([128, (n_chunks - 1) * _GI_W], _I16,
                               tag=f"giA{b}")
                gv = meta.tile([128, (n_chunks - 1) * CW], _F32,
                               tag=f"gvA{b}")
                gs = meta.tile([128, (n_chunks - 1) * CW], _F32,
                               tag=f"gsA{b}")
                gi_all.append(gi)
                gv_all.append(gv)
                gs_all.append(gs)

            for _rep in range(repeat):
                nc.sync.dma_start(out=iota_t[:], in_=iota_d[:])
                for b in range(NB):
                    nc.sync.dma_start(out=gi0[b][:],
                                      in_=gidx_d[b][:, :_GI_W])
                    nc.sync.dma_start(out=gv0[b][:], in_=gval_d[b][:, :CW])
                    nc.sync.dma_start(out=gs0[b][:], in_=gslot_d[b][:, :CW])
                for b in range(NB):
                    nc.sync.dma_start(out=gi_all[b][:],
                                      in_=gidx_d[b][:, _GI_W:])
                    nc.sync.dma_start(out=gv_all[b][:], in_=gval_d[b][:, CW:])
                    nc.sync.dma_start(out=gs_all[b][:],
                                      in_=gslot_d[b][:, CW:])
                _chunk_loop(nc, n_chunks, x_d, out_d, iota_t,
                            (gi0, gv0, gs0), gi_all, gv_all,
                            gs_all, gbuf, sbuf_s, sout, psum, opts)
    nc.compile()
    return nc


def _chunk_loop(nc, n_chunks, x_d, out_d, iota_t, meta0, gi_all, gv_all,
                gs_all, gbuf, sbuf_s, sout, psum, opts):
    gi0, gv0, gs0 = meta0
    if opts.get("sc2"):
        # timing probe: half the gather calls (2-chunk superchunks), same
        # slots; covers chunks 1..(n_chunks-1) from the resident meta tile
        for s in range((n_chunks - 1) // 2):
            for b in range(NB):
                g2_t = gbuf.tile([128, 2 * CW * EL], _F32, tag=f"g2{b}")
                nc.gpsimd.dma_gather(
                    out_ap=g2_t[:].rearrange("p (t f) -> p t f", f=EL),
                    in_ap=x_d[B_NODES * b:B_NODES * (b + 1)],
                    idxs_ap=gi_all[b][:, 2 * s * _GI_W:(2 * s + 2) * _GI_W],
                    num_idxs=2 * _NIG, num_idxs_reg=2 * _NIG, elem_size=EL,
                    single_packet=False, queue_num=b,
                )
        return
    for c in range(n_chunks):
        g_ts, s_ts = [], []
        for b in range(NB):
            if c == 0:
                gi_c = gi0[b][:]
                gv_c = gv0[b][:]
                gs_c = gs0[b][:]
            else:
                gi_c = gi_all[b][:, (c - 1) * _GI_W:c * _GI_W]
                gv_c = gv_all[b][:, (c - 1) * CW:c * CW]
                gs_c = gs_all[b][:, (c - 1) * CW:c * CW]
            g_t = gbuf.tile([128, CW * EL], _F32, tag=f"g{b}")
            if not opts.get("no_gather"):
                nc.gpsimd.dma_gather(
                    out_ap=g_t[:].rearrange("p (t f) -> p t f", f=EL),
                    in_ap=x_d[B_NODES * b:B_NODES * (b + 1)],
                    idxs_ap=gi_c,
                    num_idxs=_NIG, num_idxs_reg=_NIG, elem_size=EL,
                    single_packet=False,
                    queue_num=0 if opts.get("one_q") else b,
                )
            g_ts.append(g_t)

            s_t = sbuf_s.tile([128, CW * SEG_CAP], _F32, tag=f"s{b}")
            if not opts.get("no_dve"):
                s3 = s_t[:].rearrange("p (t s) -> p t s", s=SEG_CAP)
                gs_b = gs_c.unsqueeze(2).to_broadcast([128, CW, SEG_CAP])
                io_b = iota_t[:].unsqueeze(1).to_broadcast(
                    [128, CW, SEG_CAP])
                gv_b = gv_c.unsqueeze(2).to_broadcast([128, CW, SEG_CAP])
                nc.vector.tensor_tensor(out=s3, in0=gs_b, in1=io_b,
                                        op=mybir.AluOpType.is_equal)
                nc.vector.tensor_tensor(out=s3, in0=s3, in1=gv_b,
                                        op=mybir.AluOpType.mult)
            s_ts.append(s_t)

        # one PSUM bank-tile per 32-partition group so group a's copy+DMA
        # fire as soon as its own 40 matmuls retire (not all 120)
        for a in range(GP):
            ps = psum.tile([128, SC_H * D], _F32, space="PSUM",
                           tag=f"ps{a}")
            if not opts.get("no_pe"):
                for j in range(SC_H):
                    wl = SC_H * a + j
                    for b in range(NB):
                        nc.tensor.matmul(
                            out=ps[:SEG_CAP, D * j:D * j + D],
                            lhsT=s_ts[b][:,
                                         SEG_CAP * wl:SEG_CAP * (wl + 1)],
                            rhs=g_ts[b][:, EL * wl:EL * wl + D],
                            start=(b == 0), stop=(b == NB - 1),
                            skip_group_check=True,
                        )
            sc_t = sout.tile([128, SC_H * D], _F32, tag=f"sc{a}")
            if not opts.get("no_pe"):
                nc.scalar.copy(out=sc_t[:SEG_CAP, :], in_=ps[:SEG_CAP, :])
            if not opts.get("no_out"):
                r0 = SLOTS_PER_CHUNK * c + 320 * a
                eng = nc.sync if (c * GP + a) % 2 == 0 else nc.scalar
                eng.dma_start(
                    out=out_d[r0:r0 + 320, :].rearrange(
                        "(j s) e -> s j e", s=SEG_CAP),
                    in_=sc_t[:SEG_CAP, :].rearrange(
                        "p (j e) -> p j e", e=D),
                )


# ===========================================================================
# Entry point
# ===========================================================================
_CACHE = {}


def _get_program(n_chunks, repeat=1, opts=None):
    key = (n_chunks, repeat)
    if key not in _CACHE:
        _CACHE[key] = build_program(n_chunks, repeat, opts)
    return _CACHE[key]


def _run(adj_rows, adj_cols, adj_vals, x):
    x64 = pad_x(np.ascontiguousarray(np.asarray(x), dtype=np.float32))
    in_maps, n_chunks, pos_list = prep_inputs(adj_rows, adj_cols, adj_vals)
    for m in in_maps:
        m["x64"] = x64
    nc = _get_program(n_chunks)
    res = run_bass_kernel_spmd(nc, in_maps, core_ids=list(range(N_CORES)))
    out = np.empty((N_NODES, D), np.float32)
    for k in range(N_CORES):
        out[k * R_PER_CORE:(k + 1) * R_PER_CORE] = (
            res.results[k]["out"][pos_list[k], :])
    return out, res, (in_maps, n_chunks)


def kernel(adj_rows, adj_cols, adj_vals, x):
    out, _, _ = _run(adj_rows, adj_cols, adj_vals, x)
    return out


# revision 31
# speedup vs baseline: 1.0894x; 1.0258x over previous
"""GCN message passing (SpMM) on 8 Trainium2 NeuronCores.

out[r, :] = sum_{e: rows[e]==r} vals[e] * x[cols[e], :]

Sharding: 1D row partitioning. adj_rows is sorted, so core k owns output rows
[k*12500, (k+1)*12500) and the contiguous edge range hitting those rows.
No collectives; each core writes its own output slab.

Per-core algorithm (v5 = v4 windowed 4-bucket gather + slot-space output):
  - x is padded to [100000, 64] f32 (256B rows) and split into 4 node-range
    buckets of 25000 rows so dma_gather's int16 indices can address each.
  - Host packs output rows into "windows" (<=32 rows, <=128 edges per
    bucket per window) with a worst-fit-decreasing heuristic over the 4
    per-bucket degree dims; rows are permuted freely since the output is
    gathered from slot space host-side.  This reaches the minimal 14 chunks
    (420 windows) vs ~450 for consecutive-row grouping.  Each (window,
    bucket) is one 128-edge gather tile (tail-padded with zero-val edges).
    The last window of each chunk is kept empty when feasible; its all-pad
    tiles sit at the tail of each (bucket, chunk) index stream and are
    marked idx=-1, which dma_gather's trailing-negative trim skips at
    runtime (no descriptors generated).
  - All per-edge metadata (gather indices, vals, slot ids) is preloaded into
    SBUF once at kernel start, so the steady-state loop issues only:
    4 dma_gathers + 8 DVE ops + 120 matmuls + 1 ACT copy + 3 HWDGE output
    DMAs per 30-window chunk.
  - PE accumulates the 4 buckets' S^T @ G into one PSUM [32,48] slot per
    window => full segment sums.
  - v5 change vs v4: no dma_scatter_add.  PSUM chunk layout is chosen affine
    (window w_local = 10a + j -> psum partitions [32a,32a+32), free block j),
    so the chunk's 960 slots write to a slot-space DRAM tensor with 3 plain
    HWDGE dma_starts (zero Q7 descriptor-generation cost).  The host gathers
    row r from slot position 960*chunk + 320a + 32j + slot at unshard time
    (pure indexing).  This removes the scatter's Q7 work (~25% of runtime),
    the sidx metadata, the zero-slab preloads, and the 4-slab host sum.
  - Gathers run on SWDGE queues 0-3 (bucket b -> queue b) so descriptor
    generation uses all 4 queue contexts (8 Q7 cores); this is the kernel's
    bottleneck (~2.3ns per gather descriptor, 4 queues).
"""

import numpy as np

import concourse.bass as bass
import concourse.bacc as bacc
import concourse.mybir as mybir
import concourse.tile as tile
from concourse.bass_utils import run_bass_kernel_spmd

# ---------------- problem constants (hardcoded per the task contract) -------
N_NODES = 100000
D = 48
N_CORES = 8
R_PER_CORE = N_NODES // N_CORES  # 12500

# ---------------- kernel hyperparameters -----------------------------------
NB = 4               # node-range buckets (int16 gather indices: 25000 < 32768)
B_NODES = N_NODES // NB
EDGE_CAP = 128       # edges per (window, bucket) tile = PE contraction dim
SEG_CAP = 32         # max rows per window (= matmul M, psum partition group)
GP = 3               # usable 32-partition psum groups (offset 96 unusable)
CW = 30              # windows per chunk (= one PSUM bank: 3 groups x 10)
SC_H = CW // GP      # free blocks per psum bank (10)
EL = 128             # padded x row, bf16 elements (256B)
SLOTS_PER_CHUNK = CW * SEG_CAP  # 960

_F32 = mybir.dt.float32
_BF16 = mybir.dt.bfloat16
_BF16_NP = mybir.dt.np(mybir.dt.bfloat16)
_I16 = mybir.dt.int16

_NIG = CW * EDGE_CAP          # gather indices per (chunk, bucket) = 3840
_GI_W = _NIG // 16            # 240 int16 per partition per chunk


def _wrap16(flat, reps=8):
    """[(n)] int16 -> [16*reps, n/16] in the 16-partition wrap, replicated."""
    n = flat.shape[0]
    w = flat.reshape(n // 16, 16).T  # [16, n/16]
    return np.tile(w, (reps, 1))


# ===========================================================================
# Host-side prep: pure index/layout transformation (no float math on data).
# ===========================================================================
def _bfd_pack(deg, n_win, forbid=None):
    """Worst-fit-decreasing (LPT balancing): assign rows to n_win windows
    (<=SEG_CAP rows, per-bucket degree sum <=EDGE_CAP).  Returns
    (win_of_row, slot_of_row) or None if infeasible at this n_win.
    Windows marked in `forbid` receive no rows (kept empty so their gather
    tiles can be runtime-trimmed)."""
    r_per_core = deg.shape[0]
    cap = np.full((n_win, NB), EDGE_CAP, np.int64)
    cnt = np.zeros(n_win, np.int64)
    if forbid is not None:
        cnt[forbid] = SEG_CAP
    win_of = np.empty(r_per_core, np.int64)
    slot_of = np.empty(r_per_core, np.int64)
    order = np.argsort(-deg.sum(1), kind="stable")
    big = 1 << 40
    for r in order:
        feas = (cnt < SEG_CAP) & (cap[:, 0] >= deg[r, 0]) \
            & (cap[:, 1] >= deg[r, 1]) & (cap[:, 2] >= deg[r, 2]) \
            & (cap[:, 3] >= deg[r, 3])
        slack = cap.sum(1) - np.where(feas, 0, big)
        w = int(np.argmax(slack))
        if not feas[w]:
            return None
        win_of[r] = w
        slot_of[r] = cnt[w]
        cap[w] -= deg[r]
        cnt[w] += 1
    return win_of, slot_of


def _pack_core(rows_l, cols, vals, r_per_core, n_win_target):
    n_e = rows_l.shape[0]
    bucket = (cols // B_NODES).astype(np.int64)
    col_loc = (cols - bucket * B_NODES).astype(np.int16)

    deg = np.zeros((r_per_core, NB), np.int64)
    np.add.at(deg, (rows_l, bucket), 1)
    assert deg.max() <= EDGE_CAP, "row degree exceeds tile capacity"

    n_win = n_win_target
    spill = True
    while True:
        forbid = (np.arange(n_win) % CW == CW - 1) if spill else None
        packed = _bfd_pack(deg, n_win, forbid)
        if packed is not None:
            break
        if spill:
            spill = False       # retry same n_win without spill reservation
        else:
            n_win += CW
    window_of_row, slot_of_row = packed
    used = np.zeros(n_win, bool)
    used[window_of_row] = True

    w_e = window_of_row[rows_l]
    slot_e = slot_of_row[rows_l].astype(np.float32)

    per_bucket = []
    for b in range(NB):
        sel = np.flatnonzero(bucket == b)
        o = np.argsort(w_e[sel], kind="stable")
        sel = sel[o]
        wb = w_e[sel]                       # non-decreasing after sort
        first = np.searchsorted(wb, np.arange(n_win))
        pos = np.arange(sel.shape[0]) - first[wb]
        assert pos.max(initial=0) < EDGE_CAP
        colb = np.zeros((n_win, EDGE_CAP), np.int16)
        valb = np.zeros((n_win, EDGE_CAP), np.float32)
        slotb = np.zeros((n_win, EDGE_CAP), np.float32)
        colb[wb, pos] = col_loc[sel]
        valb[wb, pos] = vals[sel]
        slotb[wb, pos] = slot_e[sel]
        per_bucket.append((colb, valb, slotb))

    # slot-space position of each local row: window w -> chunk c=w//30,
    # w_local = w%30 = 10a + j -> pos = 960c + 320a + 32j + slot
    wl = window_of_row % CW
    pos_of_row = (SLOTS_PER_CHUNK * (window_of_row // CW)
                  + 320 * (wl // SC_H) + SEG_CAP * (wl % SC_H) + slot_of_row)
    return per_bucket, pos_of_row, n_win, used


def prep_inputs(adj_rows, adj_cols, adj_vals):
    """Shard + pack. Returns (per-core in_map list, n_chunks, pos list)."""
    adj_rows = np.asarray(adj_rows).astype(np.int64)
    adj_cols = np.asarray(adj_cols).astype(np.int64)
    adj_vals = np.asarray(adj_vals).astype(np.float32)

    bounds = np.searchsorted(adj_rows, np.arange(N_CORES + 1) * R_PER_CORE)
    packed = []
    for k in range(N_CORES):
        e0, e1 = bounds[k], bounds[k + 1]
        rows_l = adj_rows[e0:e1] - k * R_PER_CORE
        # minimal chunk count that can hold this core's edges and rows
        n_win_target = CW * max(-(-int(e1 - e0) // (NB * _NIG)),
                                -(-R_PER_CORE // (SEG_CAP * CW)))
        packed.append(_pack_core(rows_l, adj_cols[e0:e1],
                                 adj_vals[e0:e1], R_PER_CORE, n_win_target))

    nw_max = max(p[2] for p in packed)
    nw_pad = -(-nw_max // CW) * CW
    n_chunks = nw_pad // CW

    iota = np.broadcast_to(np.arange(SEG_CAP, dtype=np.float32),
                           (128, SEG_CAP)).astype(_BF16_NP)
    in_maps = []
    pos_list = []
    for k in range(N_CORES):
        per_bucket, pos_of_row, n_win, used = packed[k]
        pos_list.append(pos_of_row)
        # windows forming a trailing empty run within their chunk's gather
        # stream: mark their gather indices -1 so the ucode's trailing-
        # negative trim skips their descriptors entirely.
        used_pad = np.zeros(nw_pad, bool)
        used_pad[:n_win] = used
        trim = np.zeros(nw_pad, bool)
        for c in range(n_chunks):
            for w in range(CW * c + CW - 1, CW * c - 1, -1):
                if used_pad[w]:
                    break
                trim[w] = True
        m = {"iota": iota}
        for b in range(NB):
            colb, valb, slotb = per_bucket[b]
            cb = np.zeros((nw_pad, EDGE_CAP), np.int16)
            vb = np.zeros((nw_pad, EDGE_CAP), _BF16_NP)
            sb = np.zeros((nw_pad, EDGE_CAP), _BF16_NP)
            cb[:n_win] = colb
            vb[:n_win] = valb
            sb[:n_win] = slotb
            cb[trim] = -1
            # SBUF-resident layouts (one DMA each):
            # gidx: [128, n_chunks*_GI_W] int16 (16-wrap per chunk, x8)
            m[f"gidx{b}"] = np.concatenate([
                _wrap16(cb[c * CW:(c + 1) * CW].reshape(-1))
                for c in range(n_chunks)], axis=1)
            # vals/slot: [128, n_chunks*CW]; [p, c*CW+t] = edge t*128+p
            m[f"gval{b}"] = np.ascontiguousarray(
                vb.reshape(n_chunks, CW, EDGE_CAP).transpose(2, 0, 1)
                .reshape(128, n_chunks * CW))
            m[f"gslot{b}"] = np.ascontiguousarray(
                sb.reshape(n_chunks, CW, EDGE_CAP).transpose(2, 0, 1)
                .reshape(128, n_chunks * CW))
        in_maps.append(m)
    return in_maps, n_chunks, pos_list


def pad_x(x):
    xp = np.zeros((N_NODES, EL), _BF16_NP)
    xp[:, :D] = x.astype(_BF16_NP)
    return xp


# ===========================================================================
# Device program (shared across all 8 cores)
# ===========================================================================
def build_program(n_chunks, repeat=1, opts=None):
    opts = opts or {}
    nc = bacc.Bacc("TRN2", target_bir_lowering=False, debug=False,
                   num_devices=N_CORES, num_swdge_queues=4)
    x_d = nc.dram_tensor("x64", [N_NODES, EL], _BF16, kind="ExternalInput")
    gidx_d = [nc.dram_tensor(f"gidx{b}", [128, n_chunks * _GI_W], _I16,
                             kind="ExternalInput") for b in range(NB)]
    gval_d = [nc.dram_tensor(f"gval{b}", [128, n_chunks * CW], _BF16,
                             kind="ExternalInput") for b in range(NB)]
    gslot_d = [nc.dram_tensor(f"gslot{b}", [128, n_chunks * CW], _BF16,
                              kind="ExternalInput") for b in range(NB)]
    iota_d = nc.dram_tensor("iota", [128, SEG_CAP], _BF16,
                            kind="ExternalInput")
    out_d = nc.dram_tensor("out", [n_chunks * SLOTS_PER_CHUNK, D], _F32,
                           kind="ExternalOutput")

    with tile.TileContext(nc) as tc:
        with (
            tc.tile_pool(name="meta", bufs=1) as meta,
            tc.tile_pool(name="gbuf",
                         bufs=2 if (opts.get("shallow") or opts.get("sc2"))
                         else 3) as gbuf,
            tc.tile_pool(name="sbuf_s", bufs=3) as sbuf_s,
            tc.tile_pool(name="sout", bufs=2) as sout,
            tc.tile_pool(name="psum", bufs=2, space="PSUM") as psum,
        ):
            iota_t = meta.tile([128, SEG_CAP], _BF16)
            # chunk-0 metadata in separate small tiles so the first gathers
            # don't wait for the full metadata load
            gi0, gv0, gs0 = [], [], []
            gi_all, gv_all, gs_all = [], [], []
            for b in range(NB):
                gi0_b = meta.tile([128, _GI_W], _I16, tag=f"gi0{b}")
                gv0_b = meta.tile([128, CW], _BF16, tag=f"gv0{b}")
                gs0_b = meta.tile([128, CW], _BF16, tag=f"gs0{b}")
                gi0.append(gi0_b)
                gv0.append(gv0_b)
                gs0.append(gs0_b)
                gi = meta.tile([128, (n_chunks - 1) * _GI_W], _I16,
                               tag=f"giA{b}")
                gv = meta.tile([128, (n_chunks - 1) * CW], _BF16,
                               tag=f"gvA{b}")
                gs = meta.tile([128, (n_chunks - 1) * CW], _BF16,
                               tag=f"gsA{b}")
                gi_all.append(gi)
                gv_all.append(gv)
                gs_all.append(gs)

            for _rep in range(repeat):
                nc.sync.dma_start(out=iota_t[:], in_=iota_d[:])
                for b in range(NB):
                    nc.sync.dma_start(out=gi0[b][:],
                                      in_=gidx_d[b][:, :_GI_W])
                    nc.sync.dma_start(out=gv0[b][:], in_=gval_d[b][:, :CW])
                    nc.sync.dma_start(out=gs0[b][:], in_=gslot_d[b][:, :CW])
                for b in range(NB):
                    nc.sync.dma_start(out=gi_all[b][:],
                                      in_=gidx_d[b][:, _GI_W:])
                    nc.sync.dma_start(out=gv_all[b][:], in_=gval_d[b][:, CW:])
                    nc.sync.dma_start(out=gs_all[b][:],
                                      in_=gslot_d[b][:, CW:])
                _chunk_loop(nc, n_chunks, x_d, out_d, iota_t,
                            (gi0, gv0, gs0), gi_all, gv_all,
                            gs_all, gbuf, sbuf_s, sout, psum, opts)
    nc.compile()
    return nc


def _chunk_loop(nc, n_chunks, x_d, out_d, iota_t, meta0, gi_all, gv_all,
                gs_all, gbuf, sbuf_s, sout, psum, opts):
    gi0, gv0, gs0 = meta0
    if opts.get("sc2"):
        # timing probe: half the gather calls (2-chunk superchunks), same
        # slots; covers chunks 1..(n_chunks-1) from the resident meta tile
        for s in range((n_chunks - 1) // 2):
            for b in range(NB):
                g2_t = gbuf.tile([128, 2 * CW * EL], _F32, tag=f"g2{b}")
                nc.gpsimd.dma_gather(
                    out_ap=g2_t[:].rearrange("p (t f) -> p t f", f=EL),
                    in_ap=x_d[B_NODES * b:B_NODES * (b + 1)],
                    idxs_ap=gi_all[b][:, 2 * s * _GI_W:(2 * s + 2) * _GI_W],
                    num_idxs=2 * _NIG, num_idxs_reg=2 * _NIG, elem_size=EL,
                    single_packet=False, queue_num=b,
                )
        return
    for c in range(n_chunks):
        g_ts, s_ts = [], []
        for b in range(NB):
            if c == 0:
                gi_c = gi0[b][:]
                gv_c = gv0[b][:]
                gs_c = gs0[b][:]
            else:
                gi_c = gi_all[b][:, (c - 1) * _GI_W:c * _GI_W]
                gv_c = gv_all[b][:, (c - 1) * CW:c * CW]
                gs_c = gs_all[b][:, (c - 1) * CW:c * CW]
            g_t = gbuf.tile([128, CW * EL], _BF16, tag=f"g{b}")
            if not opts.get("no_gather"):
                nc.gpsimd.dma_gather(
                    out_ap=g_t[:].rearrange("p (t f) -> p t f", f=EL),
                    in_ap=x_d[B_NODES * b:B_NODES * (b + 1)],
                    idxs_ap=gi_c,
                    num_idxs=_NIG, num_idxs_reg=_NIG, elem_size=EL,
                    single_packet=False,
                    queue_num=0 if opts.get("one_q") else b,
                )
            g_ts.append(g_t)

            s_t = sbuf_s.tile([128, CW * SEG_CAP], _BF16, tag=f"s{b}")
            if not opts.get("no_dve"):
                s3 = s_t[:].rearrange("p (t s) -> p t s", s=SEG_CAP)
                gs_b = gs_c.unsqueeze(2).to_broadcast([128, CW, SEG_CAP])
                io_b = iota_t[:].unsqueeze(1).to_broadcast(
                    [128, CW, SEG_CAP])
                gv_b = gv_c.unsqueeze(2).to_broadcast([128, CW, SEG_CAP])
                nc.vector.tensor_tensor(out=s3, in0=gs_b, in1=io_b,
                                        op=mybir.AluOpType.is_equal)
                nc.vector.tensor_tensor(out=s3, in0=s3, in1=gv_b,
                                        op=mybir.AluOpType.mult)
            s_ts.append(s_t)

        # one PSUM bank-tile per 32-partition group so group a's copy+DMA
        # fire as soon as its own 40 matmuls retire (not all 120)
        for a in range(GP):
            ps = psum.tile([128, SC_H * D], _F32, space="PSUM",
                           tag=f"ps{a}")
            if not opts.get("no_pe"):
                nmm = 2 if opts.get("pe2x") else 1
                for j in range(SC_H):
                    wl = SC_H * a + j
                    for b in range(NB):
                        for r2 in range(nmm):
                            nc.tensor.matmul(
                                out=ps[:SEG_CAP, D * j:D * j + D],
                                lhsT=s_ts[b][:,
                                             SEG_CAP * wl:
                                             SEG_CAP * (wl + 1)],
                                rhs=g_ts[b][:, EL * wl:EL * wl + D],
                                start=(b == 0 and r2 == 0),
                                stop=(b == NB - 1 and r2 == nmm - 1),
                                skip_group_check=True,
                            )
            sc_t = sout.tile([128, SC_H * D], _F32, tag=f"sc{a}")
            if not opts.get("no_pe"):
                nc.scalar.copy(out=sc_t[:SEG_CAP, :], in_=ps[:SEG_CAP, :])
            if not opts.get("no_out"):
                r0 = SLOTS_PER_CHUNK * c + 320 * a
                eng = nc.sync if (c * GP + a) % 2 == 0 else nc.scalar
                eng.dma_start(
                    out=out_d[r0:r0 + 320, :].rearrange(
                        "(j s) e -> s j e", s=SEG_CAP),
                    in_=sc_t[:SEG_CAP, :].rearrange(
                        "p (j e) -> p j e", e=D),
                )


# ===========================================================================
# Entry point
# ===========================================================================
_CACHE = {}


def _get_program(n_chunks, repeat=1, opts=None):
    key = (n_chunks, repeat)
    if key not in _CACHE:
        _CACHE[key] = build_program(n_chunks, repeat, opts)
    return _CACHE[key]


def _run(adj_rows, adj_cols, adj_vals, x):
    x64 = pad_x(np.ascontiguousarray(np.asarray(x), dtype=np.float32))
    in_maps, n_chunks, pos_list = prep_inputs(adj_rows, adj_cols, adj_vals)
    for m in in_maps:
        m["x64"] = x64
    nc = _get_program(n_chunks)
    res = run_bass_kernel_spmd(nc, in_maps, core_ids=list(range(N_CORES)))
    out = np.empty((N_NODES, D), np.float32)
    for k in range(N_CORES):
        out[k * R_PER_CORE:(k + 1) * R_PER_CORE] = (
            res.results[k]["out"][pos_list[k], :])
    return out, res, (in_maps, n_chunks)


def kernel(adj_rows, adj_cols, adj_vals, x):
    out, _, _ = _run(adj_rows, adj_cols, adj_vals, x)
    return out
